# revision 6
# baseline (speedup 1.0000x reference)
"""Self-attention kernel for Trainium2 (Bass/Tile), 8-core SPMD — v6.

Problem: X [4, 4096, 512] f32
  S = X @ X^T per batch; W = softmax(S, -1); Y = W @ X

Sharding: batch-parallel (4 batches x 2 cores) + query-sequence parallel
(2048 queries/core, all 4096 keys), host-rolled so all 8 cores run the
same SPMD program.

Device algorithm (transposed-score layout, everything fp8 on the PE):
  - Scores computed DIRECTLY in S^T layout (keys on partitions, queries
    on the free axis) via fp8e4 DoubleRow matmuls, so the probabilities
    come out already in the layout P^T @ X needs: no probability
    transposes and no PSUM->SBUF copy traffic at all.
  - Softmax shift: exp(s_qk - m_q) with m_q = S~_qq = ||fp8(x_q)||^2
    (host-precomputed; equals the fp8-score diagonal to f32 ULPs, and
    for this data the diagonal is the row max). The shift varies along
    the FREE axis here, so it is applied as a bf16 rank-1 PSUM pass
    (ones_k (x) -m_q) accumulated after the two DoubleRow score passes;
    exp then needs no bias. Shifting by any per-row constant keeps
    softmax exact; E lands in (0, ~e^1.3] -> fits fp8.
  - exp on ACT: PSUM f32 in -> SBUF fp8 out, directly into the
    DoubleRow weight layout for PV.
  - PV: Y = E8^T @ (X8 + R8) as TWO fp8-DoubleRow passes, where
    X8 = fp8(X) and R8 = fp8(X - X8): value error ~0.4% instead of 6%.
    l = sum_k E8 from fp8-DoubleRow ones-matmuls over the same cast E8,
    so numerator/denominator quantization cancels row-wise.
  - Normalize on DVE, outputs on the Activation HWDGE queue.

Pipeline: 4 query supertiles of 512. Slot st interleaves scores(st)
with PV(st-1); the first supertile's qs0/qs1 PV trickles into slot 0
behind the exp wavefront so the prologue is never ACT-paced. The last
finalize is d-split to shorten the tail DMA chain.
"""

import ml_dtypes
import numpy as np

import concourse.bass as bass  # noqa: F401  (registers bass types)
import concourse.mybir as mybir
import concourse.tile as tile
from concourse import bacc
from concourse.bass_utils import run_bass_kernel_spmd

F32 = mybir.dt.float32
F8 = mybir.dt.float8e4
BF16 = mybir.dt.bfloat16
DR = mybir.MatmulPerfMode.DoubleRow
EXP = mybir.ActivationFunctionType.Exp

P = 128          # partitions
D = 512          # head dim
NK = 4096        # keys per batch
NQ = 2048        # queries per core
NW = 512         # score tile query-width / PSUM bank width (fp32)
KB = NK // P     # 32 key blocks per supertile column
KC2 = NK // 256  # 16 DoubleRow key chunks (PV contraction)
NST = NQ // NW   # 4 query supertiles
N_CORES = 8
B = 4

_cached = None


def _build_program():
    nc = bacc.Bacc("TRN2", target_bir_lowering=False, debug=False)
    xt8_d = nc.dram_tensor("xt8", [D, NK], F8, kind="ExternalInput").ap()
    x8_d = nc.dram_tensor("x8", [NK, D], F8, kind="ExternalInput").ap()
    r8_d = nc.dram_tensor("r8", [NK, D], F8, kind="ExternalInput").ap()
    nm_d = nc.dram_tensor("nm", [4, NQ], F8, kind="ExternalInput").ap()
    o_d = nc.dram_tensor("o", [NQ, D], F32, kind="ExternalOutput").ap()
    o_tiles = o_d.rearrange("(t p) d -> t p d", p=P)

    with tile.TileContext(nc) as tc:
        with tc.tile_pool(name="consts", bufs=1) as consts, \
             tc.tile_pool(name="e8tp", bufs=2) as e8tp, \
             tc.tile_pool(name="stats", bufs=4) as stats, \
             tc.tile_pool(name="outp", bufs=2) as outp, \
             tc.tile_pool(name="ps_s", bufs=4, space="PSUM") as ps_s, \
             tc.tile_pool(name="ps_pv", bufs=2, space="PSUM") as ps_pv, \
             tc.tile_pool(name="ps_l", bufs=2, space="PSUM") as ps_l:

            xt8_s = consts.tile([P, 4, NK], F8)      # X8^T, d on partitions
            x8_s = consts.tile([P, KC2, 2, D], F8)   # X8, k = kc2*256+i*128+p
            r8_s = consts.tile([P, KC2, 2, D], F8)   # fp8 residual of X
            # -||fp8(x_q)||^2 / 2 as an e4m3 (value, residual) pair; the
            # rank-1 shift pass contracts it against a row of 2.0s in
            # DoubleRow mode, total encoding error <= ~1.
            nm_s = consts.tile([2, 2, NQ], F8)
            twos = consts.tile([2, 2, P], F8)
            nc.vector.memset(twos, 2.0)
            ones8 = consts.tile([P, 2, 1], F8)
            nc.vector.memset(ones8, 1.0)

            # negm + r8 ride the Activation HWDGE queue (idle until the
            # first exp); xt8/x8 interleave on the SP queue so both the
            # score and PV operand streams stay ahead of compute.
            nc.scalar.dma_start(nm_s, nm_d.rearrange("(o i) q -> o i q", o=2))
            r8_r = r8_d.rearrange("(a i p) d -> p a i d", p=P, i=2)
            for g in range(4):
                nc.scalar.dma_start(
                    r8_s[:, 4 * g:4 * (g + 1)], r8_r[:, 4 * g:4 * (g + 1)])
            xt8_r = xt8_d.rearrange("(c p) n -> p c n", p=P)
            x8_r = x8_d.rearrange("(a i p) d -> p a i d", p=P, i=2)
            x8_after = {0: 0, 2: 1, 4: 2, 6: 3}
            for g in range(8):
                nc.sync.dma_start(
                    xt8_s[:, :, g * NW:(g + 1) * NW],
                    xt8_r[:, :, g * NW:(g + 1) * NW])
                if g in x8_after:
                    xg = x8_after[g]
                    nc.sync.dma_start(
                        x8_s[:, 4 * xg:4 * (xg + 1)],
                        x8_r[:, 4 * xg:4 * (xg + 1)])

            def score_tile(st, kb, et):
                """[128k, 512q] scores: 2 DR passes + bf16 rank-1 shift
                (ones_k (x) -m_q), then exp straight to fp8 E^T."""
                s_ps = ps_s.tile([P, NW], F32, name="s_ps", tag="s_ps")
                for t in range(2):
                    nc.tensor.matmul(
                        s_ps,
                        xt8_s[:, 2 * t:2 * t + 2, kb * P:(kb + 1) * P],
                        xt8_s[:, 2 * t:2 * t + 2, st * NW:(st + 1) * NW],
                        start=(t == 0), stop=False, perf_mode=DR)
                nc.tensor.matmul(
                    s_ps, twos, nm_s[:, :, st * NW:(st + 1) * NW],
                    start=False, stop=True, perf_mode=DR)
                nc.scalar.activation(et[:, kb, :], s_ps, EXP)

            def mk_state():
                return {}

            def pass_thunks(st, qs, kc2, et, state, dlo=0, dhi=D,
                            kinds=(0, 1, 2)):
                """Thunks for the PV passes of (st, qs, kc2): kind 0 = X8,
                1 = R8 (d-range [dlo,dhi)), 2 = l."""
                lhs = et[:, 2 * kc2:2 * kc2 + 2, qs * P:(qs + 1) * P]

                pvkey = "pv" if dlo == 0 else "pv2"

                def run(kind):
                    if kind == 0 and kc2 == 0 and pvkey not in state:
                        state[pvkey] = ps_pv.tile([P, D], F32, name="pv",
                                                  tag="pv")
                    if kind == 2 and "l" not in state:
                        state["l"] = ps_l.tile([P, 1], F32, name="l",
                                               tag="l")
                    if kind == 2:
                        nc.tensor.matmul(
                            state["l"], lhs, ones8,
                            start=(kc2 == 0), stop=(kc2 == KC2 - 1),
                            perf_mode=DR)
                    else:
                        rhs = (x8_s if kind == 0 else r8_s)[
                            :, kc2, :, dlo:dhi]
                        nc.tensor.matmul(
                            state[pvkey][:, dlo:dhi], lhs, rhs,
                            start=(kind == 0 and kc2 == 0),
                            stop=(kind == 1 and kc2 == KC2 - 1),
                            perf_mode=DR)
                return [(lambda k=k: run(k)) for k in kinds]

            def fin_thunk(st, qs, state, dlo=0, dhi=D):
                def run():
                    if "rl" not in state:
                        state["rl"] = stats.tile([P, 1], F32, name="rl",
                                                 tag="rl")
                        nc.vector.reciprocal(state["rl"], state["l"])
                        state["o"] = outp.tile([P, D], F32, name="o_s",
                                               tag="o_s")
                    pvkey = "pv" if dlo == 0 else "pv2"
                    nc.vector.tensor_scalar_mul(
                        state["o"][:, dlo:dhi], state[pvkey][:, dlo:dhi],
                        state["rl"])
                    nc.scalar.dma_start(
                        o_tiles[st * 4 + qs][:, dlo:dhi],
                        state["o"][:, dlo:dhi])
                return run

            # Build and emit. The fifo thunks need the actual et tile, so
            # expand lazily at emission.
            def expand(kind, st, states, et):
                thunks = []
                qs_list = {"qs01": (0, 1), "qs23": (2, 3),
                           "all": (0, 1, 2, 3)}[kind]
                last_split = (st == NST - 1)
                for kc2 in range(KC2):
                    for qs in qs_list:
                        if last_split and qs == 3:
                            # split d-halves + l-early for the tail qs
                            thunks += pass_thunks(st, qs, kc2, et,
                                                  states[qs], 0, D // 2,
                                                  (0, 1, 2))
                            continue
                        thunks += pass_thunks(st, qs, kc2, et, states[qs])
                        if kc2 == KC2 - 1:
                            thunks.append(fin_thunk(st, qs, states[qs]))
                if last_split and 3 in qs_list:
                    thunks.append(fin_thunk(st, 3, states[3], 0, D // 2))
                    for kc2 in range(KC2):
                        thunks += pass_thunks(st, 3, kc2, et, states[3],
                                              D // 2, D, (0, 1))
                    thunks.append(fin_thunk(st, 3, states[3], D // 2, D))
                return thunks

            et_tiles = {}
            all_states = {}
            for st in range(NST):
                all_states[st] = {qs: mk_state() for qs in range(4)}

            pending = []
            for slot in range(NST + 1):
                if slot < NST:
                    et_tiles[slot] = e8tp.tile([P, KB, NW], F8,
                                               name="et", tag="et")
                # assemble this slot's pv work
                items = list(pending)
                pending = []
                if slot == 1:
                    items += expand("qs23", 0, all_states[0],
                                    et_tiles[0])
                elif slot >= 2:
                    items += expand("all", slot - 1, all_states[slot - 1],
                                    et_tiles[slot - 1])
                if slot == 0:
                    slot0_items = expand("qs01", 0, all_states[0],
                                         et_tiles[0])
                    idx = 0
                    for kb in range(KB):
                        score_tile(0, kb, et_tiles[0])
                        if kb >= 4:
                            for t in slot0_items[idx:idx + 3]:
                                t()
                            idx += 3
                    pending = slot0_items[idx:]
                    continue
                if slot < NST:
                    step = -(-len(items) // KB)
                    idx = 0
                    for kb in range(KB):
                        score_tile(slot, kb, et_tiles[slot])
                        for t in items[idx:idx + step]:
                            t()
                        idx = min(idx + step, len(items))
                    for t in items[idx:]:
                        t()
                else:
                    for t in items:
                        t()

    nc.compile()
    return nc


def _get_program():
    global _cached
    if _cached is None:
        _cached = _build_program()
    return _cached


def _make_in_maps(X):
    in_maps = []
    for b in range(B):
        Xb = np.ascontiguousarray(X[b], dtype=np.float32)
        for h in range(2):
            qoff = h * NQ
            rolled = np.ascontiguousarray(np.roll(Xb, -qoff, axis=0))
            X8 = rolled.astype(ml_dtypes.float8_e4m3)
            R8 = (rolled - X8.astype(np.float32)).astype(
                ml_dtypes.float8_e4m3)
            Xq8 = X8[:NQ].astype(np.float32)
            nmh = -0.5 * np.einsum("nd,nd->n", Xq8, Xq8)
            nm0 = nmh.astype(ml_dtypes.float8_e4m3)
            nm1 = (nmh - nm0.astype(np.float32)).astype(
                ml_dtypes.float8_e4m3)
            in_maps.append({
                "xt8": np.ascontiguousarray(X8.T),
                "x8": X8,
                "r8": R8,
                "nm": np.stack([nm0, nm1,
                                np.zeros(NQ, ml_dtypes.float8_e4m3),
                                np.zeros(NQ, ml_dtypes.float8_e4m3)]),
            })
    return in_maps


def run(X, trace=False, trace_kwargs=None):
    """Run the 8-core kernel on full X [4, 4096, 512]; returns (Y, results)."""
    X = np.asarray(X)
    assert X.shape == (B, NK, D), X.shape
    nc = _get_program()
    in_maps = _make_in_maps(X)
    res = run_bass_kernel_spmd(
        nc, in_maps, core_ids=list(range(N_CORES)),
        trace=trace, **(trace_kwargs or {}))
    out = np.empty((B, NK, D), dtype=np.float32)
    for b in range(B):
        for h in range(2):
            out[b, h * NQ:(h + 1) * NQ] = res.results[2 * b + h]["o"]
    return out, res


def kernel(X):
    out, _ = run(X)
    return out


# revision 8
# speedup vs baseline: 1.0016x; 1.0016x over previous
"""Self-attention kernel for Trainium2 (Bass/Tile), 8-core SPMD — v6.

Problem: X [4, 4096, 512] f32
  S = X @ X^T per batch; W = softmax(S, -1); Y = W @ X

Sharding: batch-parallel (4 batches x 2 cores) + query-sequence parallel
(2048 queries/core, all 4096 keys), host-rolled so all 8 cores run the
same SPMD program.

Device algorithm (transposed-score layout, everything fp8 on the PE):
  - Scores computed DIRECTLY in S^T layout (keys on partitions, queries
    on the free axis) via fp8e4 DoubleRow matmuls, so the probabilities
    come out already in the layout P^T @ X needs: no probability
    transposes and no PSUM->SBUF copy traffic at all.
  - Softmax shift: exp(s_qk - m_q) with m_q = S~_qq = ||fp8(x_q)||^2
    (host-precomputed; equals the fp8-score diagonal to f32 ULPs, and
    for this data the diagonal is the row max). The shift varies along
    the FREE axis here, so it is applied as a bf16 rank-1 PSUM pass
    (ones_k (x) -m_q) accumulated after the two DoubleRow score passes;
    exp then needs no bias. Shifting by any per-row constant keeps
    softmax exact; E lands in (0, ~e^1.3] -> fits fp8.
  - exp on ACT: PSUM f32 in -> SBUF fp8 out, directly into the
    DoubleRow weight layout for PV.
  - PV: Y = E8^T @ (X8 + R8) as TWO fp8-DoubleRow passes, where
    X8 = fp8(X) and R8 = fp8(X - X8): value error ~0.4% instead of 6%.
    l = sum_k E8 from fp8-DoubleRow ones-matmuls over the same cast E8,
    so numerator/denominator quantization cancels row-wise.
  - Normalize on DVE, outputs on the Activation HWDGE queue.

Pipeline: 4 query supertiles of 512. Slot st interleaves scores(st)
with PV(st-1); the first supertile's qs0/qs1 PV trickles into slot 0
behind the exp wavefront so the prologue is never ACT-paced. The last
finalize is d-split to shorten the tail DMA chain.
"""

import ml_dtypes
import numpy as np

import concourse.bass as bass  # noqa: F401  (registers bass types)
import concourse.mybir as mybir
import concourse.tile as tile
from concourse import bacc
from concourse.bass_utils import run_bass_kernel_spmd

F32 = mybir.dt.float32
F8 = mybir.dt.float8e4
BF16 = mybir.dt.bfloat16
DR = mybir.MatmulPerfMode.DoubleRow
EXP = mybir.ActivationFunctionType.Exp

P = 128          # partitions
D = 512          # head dim
NK = 4096        # keys per batch
NQ = 2048        # queries per core
NW = 512         # score tile query-width / PSUM bank width (fp32)
KB = NK // P     # 32 key blocks per supertile column
KC2 = NK // 256  # 16 DoubleRow key chunks (PV contraction)
NST = NQ // NW   # 4 query supertiles
N_CORES = 8
B = 4

_cached = None


def _build_program():
    nc = bacc.Bacc("TRN2", target_bir_lowering=False, debug=False)
    xt8_d = nc.dram_tensor("xt8", [D, NK], F8, kind="ExternalInput").ap()
    x8_d = nc.dram_tensor("x8", [NK, D], F8, kind="ExternalInput").ap()
    r8_d = nc.dram_tensor("r8", [NK, D], F8, kind="ExternalInput").ap()
    nm_d = nc.dram_tensor("nm", [4, NQ], F8, kind="ExternalInput").ap()
    o_d = nc.dram_tensor("o", [NQ, D], F32, kind="ExternalOutput").ap()
    o_tiles = o_d.rearrange("(t p) d -> t p d", p=P)

    with tile.TileContext(nc) as tc:
        with tc.tile_pool(name="consts", bufs=1) as consts, \
             tc.tile_pool(name="e8tp", bufs=2) as e8tp, \
             tc.tile_pool(name="stats", bufs=4) as stats, \
             tc.tile_pool(name="outp", bufs=2) as outp, \
             tc.tile_pool(name="ps_s", bufs=2, space="PSUM") as ps_s, \
             tc.tile_pool(name="ps_pv", bufs=2, space="PSUM") as ps_pv, \
             tc.tile_pool(name="ps_l", bufs=2, space="PSUM") as ps_l:

            xt8_s = consts.tile([P, 4, NK], F8)      # X8^T, d on partitions
            x8_s = consts.tile([P, KC2, 2, D], F8)   # X8, k = kc2*256+i*128+p
            r8_s = consts.tile([P, KC2, 2, D], F8)   # fp8 residual of X
            # -||fp8(x_q)||^2 / 2 as an e4m3 (value, residual) pair; the
            # rank-1 shift pass contracts it against a row of 2.0s in
            # DoubleRow mode, total encoding error <= ~1.
            nm_s = consts.tile([2, 2, NQ], F8)
            twos = consts.tile([2, 2, P], F8)
            nc.vector.memset(twos, 2.0)
            ones8 = consts.tile([P, 2, 1], F8)
            nc.vector.memset(ones8, 1.0)

            # negm + r8 ride the Activation HWDGE queue (idle until the
            # first exp); xt8/x8 interleave on the SP queue so both the
            # score and PV operand streams stay ahead of compute.
            nc.scalar.dma_start(nm_s, nm_d.rearrange("(o i) q -> o i q", o=2))
            r8_r = r8_d.rearrange("(a i p) d -> p a i d", p=P, i=2)
            for g in range(4):
                nc.scalar.dma_start(
                    r8_s[:, 4 * g:4 * (g + 1)], r8_r[:, 4 * g:4 * (g + 1)])
            xt8_r = xt8_d.rearrange("(c p) n -> p c n", p=P)
            x8_r = x8_d.rearrange("(a i p) d -> p a i d", p=P, i=2)
            x8_after = {0: 0, 2: 1, 4: 2, 6: 3}
            for g in range(8):
                nc.sync.dma_start(
                    xt8_s[:, :, g * NW:(g + 1) * NW],
                    xt8_r[:, :, g * NW:(g + 1) * NW])
                if g in x8_after:
                    xg = x8_after[g]
                    nc.sync.dma_start(
                        x8_s[:, 4 * xg:4 * (xg + 1)],
                        x8_r[:, 4 * xg:4 * (xg + 1)])

            def score_tile(st, kb, et):
                """[128k, 512q] scores: 2 DR passes + bf16 rank-1 shift
                (ones_k (x) -m_q), then exp straight to fp8 E^T."""
                s_ps = ps_s.tile([P, NW], F32, name="s_ps", tag="s_ps")
                for t in range(2):
                    nc.tensor.matmul(
                        s_ps,
                        xt8_s[:, 2 * t:2 * t + 2, kb * P:(kb + 1) * P],
                        xt8_s[:, 2 * t:2 * t + 2, st * NW:(st + 1) * NW],
                        start=(t == 0), stop=False, perf_mode=DR)
                nc.tensor.matmul(
                    s_ps, twos, nm_s[:, :, st * NW:(st + 1) * NW],
                    start=False, stop=True, perf_mode=DR)
                nc.scalar.activation(et[:, kb, :], s_ps, EXP)

            def mk_state():
                return {}

            def pass_thunks(st, qs, kc2, et, state, dlo=0, dhi=D,
                            kinds=(0, 1, 2)):
                """Thunks for the PV passes of (st, qs, kc2): kind 0 = X8,
                1 = R8 (d-range [dlo,dhi)), 2 = l."""
                lhs = et[:, 2 * kc2:2 * kc2 + 2, qs * P:(qs + 1) * P]

                pvkey = "pv" if dlo == 0 else "pv2"

                def run(kind):
                    if kind == 0 and kc2 == 0 and pvkey not in state:
                        state[pvkey] = ps_pv.tile([P, D], F32, name="pv",
                                                  tag="pv")
                    if kind == 2 and "l" not in state:
                        state["l"] = ps_l.tile([P, 1], F32, name="l",
                                               tag="l")
                    if kind == 2:
                        nc.tensor.matmul(
                            state["l"], lhs, ones8,
                            start=(kc2 == 0), stop=(kc2 == KC2 - 1),
                            perf_mode=DR)
                    else:
                        rhs = (x8_s if kind == 0 else r8_s)[
                            :, kc2, :, dlo:dhi]
                        nc.tensor.matmul(
                            state[pvkey][:, dlo:dhi], lhs, rhs,
                            start=(kind == 0 and kc2 == 0),
                            stop=(kind == 1 and kc2 == KC2 - 1),
                            perf_mode=DR)
                return [(lambda k=k: run(k)) for k in kinds]

            def fin_thunk(st, qs, state, dlo=0, dhi=D):
                def run():
                    if "rl" not in state:
                        state["rl"] = stats.tile([P, 1], F32, name="rl",
                                                 tag="rl")
                        nc.vector.reciprocal(state["rl"], state["l"])
                        state["o"] = outp.tile([P, D], F32, name="o_s",
                                               tag="o_s")
                    pvkey = "pv" if dlo == 0 else "pv2"
                    nc.vector.tensor_scalar_mul(
                        state["o"][:, dlo:dhi], state[pvkey][:, dlo:dhi],
                        state["rl"])
                    nc.scalar.dma_start(
                        o_tiles[st * 4 + qs][:, dlo:dhi],
                        state["o"][:, dlo:dhi])
                return run

            # Build and emit. The fifo thunks need the actual et tile, so
            # expand lazily at emission.
            def expand(kind, st, states, et):
                thunks = []
                qs_list = {"qs01": (0, 1), "qs23": (2, 3),
                           "all": (0, 1, 2, 3)}[kind]
                last_split = (st == NST - 1)
                for kc2 in range(KC2):
                    for qs in qs_list:
                        if last_split and qs == 3:
                            # split d-halves + l-early for the tail qs
                            thunks += pass_thunks(st, qs, kc2, et,
                                                  states[qs], 0, D // 2,
                                                  (0, 1, 2))
                            continue
                        thunks += pass_thunks(st, qs, kc2, et, states[qs])
                        if kc2 == KC2 - 1:
                            thunks.append(fin_thunk(st, qs, states[qs]))
                if last_split and 3 in qs_list:
                    thunks.append(fin_thunk(st, 3, states[3], 0, D // 2))
                    for kc2 in range(KC2):
                        thunks += pass_thunks(st, 3, kc2, et, states[3],
                                              D // 2, D, (0, 1))
                    thunks.append(fin_thunk(st, 3, states[3], D // 2, D))
                return thunks

            et_tiles = {}
            all_states = {}
            for st in range(NST):
                all_states[st] = {qs: mk_state() for qs in range(4)}

            pending = []
            for slot in range(NST + 1):
                if slot < NST:
                    et_tiles[slot] = e8tp.tile([P, KB, NW], F8,
                                               name="et", tag="et")
                # assemble this slot's pv work
                items = list(pending)
                pending = []
                if slot == 1:
                    items += expand("qs23", 0, all_states[0],
                                    et_tiles[0])
                elif slot >= 2:
                    items += expand("all", slot - 1, all_states[slot - 1],
                                    et_tiles[slot - 1])
                if slot == 0:
                    slot0_items = expand("qs01", 0, all_states[0],
                                         et_tiles[0])
                    idx = 0
                    for kb in range(KB):
                        score_tile(0, kb, et_tiles[0])
                        if kb >= 4:
                            for t in slot0_items[idx:idx + 3]:
                                t()
                            idx += 3
                    pending = slot0_items[idx:]
                    continue
                if slot < NST:
                    step = -(-len(items) // KB)
                    idx = 0
                    for kb in range(KB):
                        score_tile(slot, kb, et_tiles[slot])
                        for t in items[idx:idx + step]:
                            t()
                        idx = min(idx + step, len(items))
                    for t in items[idx:]:
                        t()
                else:
                    for t in items:
                        t()

    nc.compile()
    return nc


def _get_program():
    global _cached
    if _cached is None:
        _cached = _build_program()
    return _cached


def _make_in_maps(X):
    in_maps = []
    for b in range(B):
        Xb = np.ascontiguousarray(X[b], dtype=np.float32)
        for h in range(2):
            qoff = h * NQ
            rolled = np.ascontiguousarray(np.roll(Xb, -qoff, axis=0))
            X8 = rolled.astype(ml_dtypes.float8_e4m3)
            R8 = (rolled - X8.astype(np.float32)).astype(
                ml_dtypes.float8_e4m3)
            Xq8 = X8[:NQ].astype(np.float32)
            nmh = -0.5 * np.einsum("nd,nd->n", Xq8, Xq8)
            nm0 = nmh.astype(ml_dtypes.float8_e4m3)
            nm1 = (nmh - nm0.astype(np.float32)).astype(
                ml_dtypes.float8_e4m3)
            in_maps.append({
                "xt8": np.ascontiguousarray(X8.T),
                "x8": X8,
                "r8": R8,
                "nm": np.stack([nm0, nm1,
                                np.zeros(NQ, ml_dtypes.float8_e4m3),
                                np.zeros(NQ, ml_dtypes.float8_e4m3)]),
            })
    return in_maps


def run(X, trace=False, trace_kwargs=None):
    """Run the 8-core kernel on full X [4, 4096, 512]; returns (Y, results)."""
    X = np.asarray(X)
    assert X.shape == (B, NK, D), X.shape
    nc = _get_program()
    in_maps = _make_in_maps(X)
    res = run_bass_kernel_spmd(
        nc, in_maps, core_ids=list(range(N_CORES)),
        trace=trace, **(trace_kwargs or {}))
    out = np.empty((B, NK, D), dtype=np.float32)
    for b in range(B):
        for h in range(2):
            out[b, h * NQ:(h + 1) * NQ] = res.results[2 * b + h]["o"]
    return out, res


def kernel(X):
    out, _ = run(X)
    return out
